# revision 4
# baseline (speedup 1.0000x reference)
"""Multi-head attention on 8 Trainium2 NeuronCores (head-parallel), v3.

Problem: Q,K,V [4096,512] fp32; Wq/Wk/Wv [8,512,64]; Wo [512,512].
  out = concat_h(softmax(QWq_h (KWk_h)^T / sqrt(64)) VWv_h) @ Wo

Sharding: one head per core. Each core computes its head end-to-end plus
its slice of the output projection (out_h @ Wo[64h:64h+64, :]); the host
sums the 8 partial [4096,512] outputs.

Numerics: fp16 parts are exactly representable to the PE, so fp16
matmuls are exact on them; splitting each operand hi/lo (fp16 pair =
~22 mantissa bits) makes every precision-critical product exact up to a
dropped lo*lo term (~2^-22 relative):
  - projections: 3-term fp16 (hi*hi + lo*hi + hi*lo), 12 chunked
    matmuls per 512-column chunk;
  - scores: TWO matmuls per m-tile: hi*hi (K=65, with k row 64 = -1 and
    q row 64 = rowmax so PSUM = qk^T - rowmax directly) plus one
    contraction-stacked cross term (K=128: [kl;kh]^T [qh;ql] =
    kl*qh + kh*ql), instead of the naive three;
  - stats (rowmax estimate): hi*hi only, reduced on DVE; the +-40 error
    (dropped cross + fp16 rowmax storage) sits far inside exp()'s
    overflow budget and cancels exactly via the denominator;
  - attn.V in bf16 with a ones column producing the softmax denominator
    in row d; Wo with float32r operands (tf32-rounded, fine for the
    uniform-positive Wo).

Schedule: only q chunk 0 + K are needed before era 0, so the remaining
q-chunk projections, v projections, and next-era stats are threaded into
era groups (era_work); V/Q bulk DMAs are ordered so nothing stalls the
SP queue or the one-FIFO DMA device ahead of a dependency.
"""

from contextlib import ExitStack

import numpy as np

N = 4096
DIM = 512
H = 8
D = 64
P = 128
CH = 512  # query columns per era (chunk)


def build_head_kernel(ctx, tc, outs, ins, n=N, dim=DIM, d=D):
    import concourse.bass as bass
    import concourse.mybir as mybir
    from concourse.bass import ts, ds

    nc = tc.nc
    f32 = mybir.dt.float32
    f32r = mybir.dt.float32r
    bf16 = mybir.dt.bfloat16
    f16 = mybir.dt.float16
    AF = mybir.ActivationFunctionType

    KC = dim // P      # projection contraction chunks (4)
    NT = n // P        # 128-row tiles of n (= m tiles) (32)
    NCH = n // CH      # eras (8)
    NTC = CH // P      # n-tiles per era (4)
    GRP = NT // 2      # main groups per era, 2 m-tiles each (16)
    SPAIR = n // 512   # stats m-chunks per q-tile (8)
    SITEMS = NTC * SPAIR  # stats items per era (32)
    NB = n // 512      # input chunks (8)
    assert n % 1024 == 0 and dim % P == 0 and CH == 512

    qth_d, qtl_d = ins["QTH"], ins["QTL"]
    kth_d, ktl_d = ins["KTH"], ins["KTL"]
    vt_d = ins["VT"]
    wqh_d, wql_d = ins["wqh"], ins["wql"]
    wkh_d, wkl_d = ins["wkh"], ins["wkl"]
    wv_d, wo_d = ins["wv"], ins["wo"]
    cneg_d, cone_d = ins["cneg"], ins["cone"]
    out_d = outs["out"]
    den_d = outs["den"]

    singles = ctx.enter_context(tc.tile_pool(name="singles", bufs=1))

    # Persistent SBUF tensors.
    At_ev = singles.tile([d + 1, n], f16)  # q hi; row d = rowmax (even eras)
    At_od = singles.tile([d + 1, n], f16)  # q hi; row d = rowmax (odd eras)
    Kt = singles.tile([d + 1, n], f16)     # k hi; row d = -1
    Qc = singles.tile([P, n], f16)         # [q hi; q lo] for the cross term
    Kc = singles.tile([P, n], f16)         # [k lo; k hi] for the cross term
    v_sb = singles.tile([P, NT, d + 1], bf16)  # v tiles + ones column
    outT = singles.tile([d, n], f32r)      # attn_u @ v
    rsum = singles.tile([P, NT], f32)      # sumexp gathered per n-tile
    wqh_sb = singles.tile([P, KC, d], f16)
    wql_sb = singles.tile([P, KC, d], f16)
    wkh_sb = singles.tile([P, KC, d], f16)
    wkl_sb = singles.tile([P, KC, d], f16)
    wv_sb = singles.tile([P, KC, d], bf16)
    wo_sb = singles.tile([d, dim], f32r)

    def _load_w(w_sb, w_d):
        nc.sync.dma_start(out=w_sb, in_=w_d.rearrange("(c p) e -> p c e", p=P))

    # stats PSUM pool opens before P1 so chunk-0 stats can run inside the
    # projection phase as its k-chunks land; it also provides the wo and
    # projection PSUM tiles (identical shape/tag, never alive concurrently
    # within a buffer slot).
    st_pool = ctx.enter_context(tc.tile_pool(name="st_ps_pool", bufs=3, space="PSUM"))
    nmax_pool = ctx.enter_context(tc.tile_pool(name="nmax_pool", bufs=5))
    lo_pool = ctx.enter_context(tc.tile_pool(name="lo_pool", bufs=3))

    nmax_tiles = {}

    def stats_item(c, g, ps_pool=None):
        """One stats item: q-tile j of chunk c vs a 512-wide m-chunk p, as
        one fp16 hi*hi matmul + one DVE max-reduce."""
        p, j = divmod(g, NTC)
        st = (ps_pool or st_pool).tile([P, 512], f32, tag="st")
        nc.tensor.matmul(st, lhsT=At_ev[0:d, ds(c * CH + j * P, P)],
                         rhs=Kt[0:d, ds(512 * p, 512)],
                         start=True, stop=True)
        if p == 0:
            nmax_tiles[j] = nmax_pool.tile([P, SPAIR], f16, tag="nmax",
                                           name="nmax")
        nc.vector.reduce_max(nmax_tiles[j][:, p:p + 1], st,
                             axis=mybir.AxisListType.X)
        if p == SPAIR - 1:
            if j == 0:
                stats_item.cmax = nmax_pool.tile([P, NTC], f16, tag="cmax")
            nc.vector.reduce_max(stats_item.cmax[:, j:j + 1], nmax_tiles[j],
                                 axis=mybir.AxisListType.X)
            At = At_ev if c % 2 == 0 else At_od
            nc.sync.dma_start(out=At[d:d + 1, ds(c * CH + j * P, P)],
                              in_=stats_item.cmax[:, j:j + 1])

    # ---- P1: projections (3-term fp16), chunk-0 stats folded in ----
    pstream = ctx.enter_context(tc.tile_pool(name="pstream", bufs=3))
    qstream = ctx.enter_context(tc.tile_pool(name="qstream", bufs=4))
    vstream = ctx.enter_context(tc.tile_pool(name="vstream", bufs=3))

    def qk_work(nb, th, tl, wh, wl, hi65, cdst, is_q, od_dup, ps_pool=None):
        """Project one 512-col chunk (3-term fp16) and evict the hi part
        (into the K=65 operand) and lo part (into the cross operand)."""
        nbs = ds(nb * 512, 512)
        ps = (ps_pool or st_pool).tile([P, 512], f32, tag="st",
                                       name="ps_qk")[0:d, :]
        terms = [(wh, th), (wh, tl), (wl, th)]
        for i, (w, t) in enumerate(terms):
            for kc in range(KC):
                nc.tensor.matmul(ps, lhsT=w[:, kc, :], rhs=t[:, kc, :],
                                 start=(i == 0 and kc == 0),
                                 stop=(i == 2 and kc == KC - 1))
        nc.scalar.copy(hi65[0:d, nbs], ps)          # hi = f16(x)
        lo_t = lo_pool.tile([d, 512], f16, tag="lo")
        nc.vector.tensor_sub(lo_t, ps, hi65[0:d, nbs])  # lo = x - hi
        if is_q:
            # Qc = [q hi (direct copy); q lo (DMA to upper partitions)]
            nc.vector.tensor_copy(cdst[0:d, nbs], hi65[0:d, nbs])
            nc.sync.dma_start(out=cdst[d:2 * d, nbs], in_=lo_t)
        else:
            # Kc = [k lo (direct); k hi (DMA to upper partitions)]
            nc.vector.tensor_copy(cdst[0:d, nbs], lo_t)
            nc.sync.dma_start(out=cdst[d:2 * d, nbs], in_=hi65[0:d, nbs])
        if od_dup:
            # odd-era copy of q hi (separate tile so era c+1's rowmax
            # scatter never WARs era c's score matmul reads)
            nc.sync.dma_start(out=At_od[0:d, nbs], in_=hi65[0:d, nbs])

    def v_work(vb, vt_t):
        """Project 4 v m-tiles from one loaded VT chunk as a single PSUM
        accumulation group (one zero-region), evicted by one DVE copy."""
        ps_v = st_pool.tile([P, 512], f32, tag="st", name="ps_v")
        for i in range(4):
            for kc in range(KC):
                nc.tensor.matmul(ps_v[:, 64 * i:64 * i + 64],
                                 lhsT=vt_t[:, kc, ts(i, P)],
                                 rhs=wv_sb[:, kc, :],
                                 start=(i == 0 and kc == 0),
                                 stop=(i == 3 and kc == KC - 1))
        nc.vector.tensor_copy(v_sb[:, ds(4 * vb, 4), 0:d], ps_v[:, 0:256])

    def load_chunk(pool, t_d, tag, dtype, nb):
        t = pool.tile([P, KC, 512], dtype, tag=tag, name=tag)
        src_ap = t_d[:, ds(nb * 512, 512)].rearrange("(c p) x -> p c x", p=P)
        half = KC // 2
        nc.sync.dma_start(out=t[:, 0:half, :], in_=src_ap[:, 0:half, :])
        nc.sync.dma_start(out=t[:, half:KC, :], in_=src_ap[:, half:KC, :])
        return t

    def load_qk(pool, hi_d, lo_d, tag, nb):
        return (load_chunk(pool, hi_d, tag + "h", f16, nb),
                load_chunk(pool, lo_d, tag + "l", f16, nb))

    _load_w(wqh_sb, wqh_d)
    _load_w(wql_sb, wql_d)
    qt_tiles = {0: load_qk(qstream, qth_d, qtl_d, "qt", 0)}
    _load_w(wkh_sb, wkh_d)
    _load_w(wkl_sb, wkl_d)
    nc.sync.dma_start(out=Kt[d:d + 1, :], in_=cneg_d)
    nc.sync.dma_start(out=v_sb[:, :, d:d + 1],
                      in_=cone_d.rearrange("p (t o) -> p t o", o=1))

    with tc.tile_pool(name="p1_ps", bufs=5, space="PSUM") as p1_pool:
        qk_work(0, *qt_tiles[0], wqh_sb, wql_sb, At_ev, Qc, True, False,
                ps_pool=p1_pool)
        for p in range(NB):
            kt_t = load_qk(pstream, kth_d, ktl_d, "kt", p)
            if p == 0:
                _load_w(wv_sb, wv_d)
                nc.sync.dma_start(out=wo_sb, in_=wo_d)
            qk_work(p, *kt_t, wkh_sb, wkl_sb, Kt, Kc, False, False,
                    ps_pool=p1_pool)
            # chunk-0 stats lag one k-chunk so they never wait on the
            # freshly-evicted hi part (its ACT copy is long done)
            if p >= 1:
                for j in range(NTC):
                    stats_item(0, (p - 1) * NTC + j, ps_pool=p1_pool)
        for j in range(NTC):
            stats_item(0, (NB - 1) * NTC + j, ps_pool=p1_pool)

    qt_tiles[1] = load_qk(qstream, qth_d, qtl_d, "qt", 1)
    # q chunk 1 projected before era 0 so its stats can spread over the
    # whole of era 0; its small DMAs land right after qt1.
    qk_work(1, *qt_tiles[1], wqh_sb, wql_sb, At_ev, Qc, True, True)

    vt_tiles = {}
    for item in ("v0", "v1", "v2", "v3", "v4", "v5", "v6", "v7", "q2", "q3"):
        idx = int(item[1:])
        if item[0] == "v":
            vt_tiles[idx] = load_chunk(vstream, vt_d, "vt", bf16, idx)
        else:
            qt_tiles[idx] = load_qk(qstream, qth_d, qtl_d, "qt", idx)

    # Work queue threaded into era groups: (era, group) -> list of thunks.
    era_work = {}

    def add_work(c, g, fn):
        era_work.setdefault((c, g), []).append(fn)

    for vb in range(8):
        add_work(0, 2 * vb + 1, lambda vb=vb: v_work(vb, vt_tiles[vb]))
    add_work(0, 2, lambda: qt_tiles.update(
        {4: load_qk(qstream, qth_d, qtl_d, "qt", 4)}))
    add_work(1, 0, lambda: qt_tiles.update(
        {5: load_qk(qstream, qth_d, qtl_d, "qt", 5)}))
    add_work(2, 0, lambda: qt_tiles.update(
        {6: load_qk(qstream, qth_d, qtl_d, "qt", 6)}))
    add_work(3, 0, lambda: qt_tiles.update(
        {7: load_qk(qstream, qth_d, qtl_d, "qt", 7)}))
    QK_SLOT = {2: (0, 10), 3: (1, 2), 4: (2, 2), 5: (3, 2), 6: (4, 2),
               7: (5, 2)}
    for qb in range(2, NB):
        c, g = QK_SLOT[qb]
        add_work(c, g, lambda qb=qb: qk_work(
            qb, *qt_tiles[qb], wqh_sb, wql_sb, At_ev, Qc, True, qb % 2 == 1))
    # stats for chunk c+1 threaded into era c (era 0's q-chunk-1 inputs are
    # ready before it starts, so every era spreads them over groups 0..13)
    for c in range(NCH - 1):
        for k in range(SITEMS):
            g = (k * 14) // SITEMS
            fn = lambda c=c, k=k: stats_item(c + 1, k)
            fn.is_stats = True
            add_work(c, g, fn)

    # ---- P2: eras ----
    sc_pool = ctx.enter_context(tc.tile_pool(name="sc_ps_pool", bufs=2, space="PSUM"))
    av_pool = ctx.enter_context(tc.tile_pool(name="av_ps_pool", bufs=1, space="PSUM"))
    att_pool = ctx.enter_context(tc.tile_pool(name="att_pool", bufs=3))
    o_pool = ctx.enter_context(tc.tile_pool(name="o_pool", bufs=4))

    def wo_tile(t):
        """Output-projection for n-tile t, scaled by 1/sumexp on eviction."""
        ps = st_pool.tile([P, 512], f32, tag="st", name="wops")
        nc.tensor.matmul(ps, lhsT=outT[:, ts(t, P)], rhs=wo_sb,
                         start=True, stop=True)
        o_sb = o_pool.tile([P, dim], f32, tag="o_sb")
        # unnormalized eviction: the 1/sumexp division happens on the host
        # (which already sums the 8 per-head partials), keeping the
        # denominator chain off the device's critical path
        if t % 2 == 0:
            nc.scalar.copy(o_sb, ps)
        else:
            nc.vector.tensor_copy(o_sb, ps)
        nc.sync.dma_start(out=out_d[ts(t, P), :], in_=o_sb)

    def era(c):
        At = At_ev if c % 2 == 0 else At_od
        cs = ds(c * CH, CH)
        r65 = At[:, cs]   # [d+1, 512], row d = rowmax
        rc = Qc[:, cs]    # [128, 512] = [q hi; q lo]
        av_ps = av_pool.tile([d + 1, CH], f32, tag="av")
        att_fifo = []  # (att_tile, g) awaiting attn.V, deferred 2 groups

        def emit_av(att_t, g):
            nc.tensor.matmul(av_ps, lhsT=v_sb[:, 2 * g, :], rhs=att_t[:, 0:512],
                             start=(g == 0), stop=False)
            nc.tensor.matmul(av_ps, lhsT=v_sb[:, 2 * g + 1, :],
                             rhs=att_t[:, 512:1024],
                             start=False, stop=(g == GRP - 1))

        for g in range(GRP):
            work = list(era_work.get((c, g), ()))
            stats_fns = [f for f in work if getattr(f, "is_stats", False)]
            other_fns = [f for f in work if not getattr(f, "is_stats", False)]
            if stats_fns:
                stats_fns[0]()
            mta, mtb = ts(2 * g, P), ts(2 * g + 1, P)
            sc_ps = sc_pool.tile([P, 1024], f32, tag="sc")
            att_t = att_pool.tile([P, 1024], bf16, tag="att")
            # scores^T - rowmax in two exact fp16 matmuls per m-tile:
            # hi*hi (K=65 with the -1/rowmax row) + stacked cross (K=128)
            nc.tensor.matmul(sc_ps[:, 0:512], lhsT=Kt[:, mta], rhs=r65,
                             start=True, stop=False)
            nc.tensor.matmul(sc_ps[:, 0:512], lhsT=Kc[:, mta], rhs=rc,
                             start=False, stop=True)
            nc.tensor.matmul(sc_ps[:, 512:1024], lhsT=Kt[:, mtb], rhs=r65,
                             start=True, stop=False)
            nc.tensor.matmul(sc_ps[:, 512:1024], lhsT=Kc[:, mtb], rhs=rc,
                             start=False, stop=True)
            for f in stats_fns[1:]:
                f()
            for f in other_fns:
                f()
            nc.scalar.activation(att_t, sc_ps, AF.Exp)
            # defer attn.V two groups so the exp it reads is long done
            att_fifo.append((att_t, g))
            if len(att_fifo) > 2:
                emit_av(*att_fifo.pop(0))
            # wo for the previous era's tiles, on the stats PSUM pool
            if c > 0 and g in (1, 5, 9, 13):
                wo_tile((c - 1) * NTC + (g - 1) // 4)
        for item in att_fifo:
            emit_av(*item)
        # evict attn_u @ v and the sumexp row, then gather the per-n-tile
        # denominators
        sx = o_pool.tile([1, CH], f32, tag="sx")
        if c == NCH - 1:
            # last era: rinv gates the tail wo; DVE is free by now
            nc.vector.tensor_copy(sx, av_ps[d:d + 1, :])
        else:
            nc.scalar.copy(sx, av_ps[d:d + 1, :])
        nc.scalar.copy(outT[:, cs], av_ps[0:d, :])
        for jj in range(NTC):
            # last era: split the gather issue across SP (HWDGE) and GpSimd
            # (SWDGE) so the four small DMAs don't serialize on one issuer
            eng = nc.gpsimd if (c == NCH - 1 and jj % 2 == 1) else nc.sync
            eng.dma_start(out=rsum[:, c * NTC + jj:c * NTC + jj + 1],
                          in_=sx[:, ds(jj * P, P)])
        if c == NCH - 1:
            nc.sync.dma_start(out=den_d, in_=rsum)

    for c in range(NCH):
        era(c)

    # ---- P3 tail: last era's output projection ----
    for t in range(NT - NTC, NT):
        wo_tile(t)


def make_in_maps(Q, K, V, Wq, Wk, Wv, Wo):
    """Host-side sharding: transpose activations, fp16 hi/lo planes, slice
    weights per head."""
    import ml_dtypes

    def hilo16(x):
        hi = x.astype(np.float16)
        lo = (x - hi.astype(np.float32)).astype(np.float16)
        return np.ascontiguousarray(hi), np.ascontiguousarray(lo)

    scale = 1.0 / np.sqrt(Wq.shape[-1])
    QTH, QTL = hilo16(np.ascontiguousarray(Q.T.astype(np.float32)))
    KTH, KTL = hilo16(np.ascontiguousarray(K.T.astype(np.float32)))
    VT = np.ascontiguousarray(V.T.astype(np.float32).astype(ml_dtypes.bfloat16))
    d = Wq.shape[-1]
    n = Q.shape[0]
    in_maps = []
    for h in range(Wq.shape[0]):
        wqh, wql = hilo16(Wq[h].astype(np.float32) * scale)
        wkh, wkl = hilo16(Wk[h].astype(np.float32))
        in_maps.append({
            "QTH": QTH, "QTL": QTL, "KTH": KTH, "KTL": KTL, "VT": VT,
            "wqh": wqh, "wql": wql, "wkh": wkh, "wkl": wkl,
            "wv": np.ascontiguousarray(
                Wv[h].astype(np.float32).astype(ml_dtypes.bfloat16)),
            "wo": np.ascontiguousarray(Wo[h * d:(h + 1) * d, :].astype(np.float32)),
            "cneg": np.full((1, n), -1.0, np.float16),
            "cone": np.ones((128, n // 128), ml_dtypes.bfloat16),
        })
    return in_maps


_CACHE = {}


def _build_and_compile(n=N, dim=DIM, d=D, num_cores=H, repeats=1):
    import concourse.bass as bass
    import concourse.mybir as mybir
    import concourse.tile as tile
    from concourse import bacc

    key = (n, dim, d, num_cores, repeats)
    if key in _CACHE:
        return _CACHE[key]
    nc = bacc.Bacc("TRN2", target_bir_lowering=False, debug=False,
                   num_devices=num_cores)
    f32 = mybir.dt.float32
    f32r = mybir.dt.float32r
    bf16 = mybir.dt.bfloat16
    f16 = mybir.dt.float16
    ins = {}
    for name in ("QTH", "QTL", "KTH", "KTL"):
        ins[name] = nc.dram_tensor(name, [dim, n], f16, kind="ExternalInput").ap()
    ins["VT"] = nc.dram_tensor("VT", [dim, n], bf16, kind="ExternalInput").ap()
    for name in ("wqh", "wql", "wkh", "wkl"):
        ins[name] = nc.dram_tensor(name, [dim, d], f16, kind="ExternalInput").ap()
    ins["wv"] = nc.dram_tensor("wv", [dim, d], bf16, kind="ExternalInput").ap()
    ins["wo"] = nc.dram_tensor("wo", [d, dim], f32r, kind="ExternalInput").ap()
    ins["cneg"] = nc.dram_tensor("cneg", [1, n], f16, kind="ExternalInput").ap()
    ins["cone"] = nc.dram_tensor("cone", [128, n // 128], bf16,
                                 kind="ExternalInput").ap()
    outs = {"out": nc.dram_tensor("out", [n, dim], f32, kind="ExternalOutput").ap(),
            "den": nc.dram_tensor("den", [128, n // 128], f32,
                                  kind="ExternalOutput").ap()}
    with tile.TileContext(nc) as tc:
        for _rep in range(repeats):
            with ExitStack() as ctx:
                build_head_kernel(ctx, tc, outs, ins, n=n, dim=dim, d=d)
    nc.compile()
    _CACHE[key] = nc
    return nc


def run_on_hw(in_maps, trace=False, **kwargs):
    from concourse.bass_utils import run_bass_kernel_spmd

    nc = _build_and_compile(num_cores=len(in_maps))
    return run_bass_kernel_spmd(nc, in_maps, core_ids=list(range(len(in_maps))),
                                trace=trace, **kwargs)


def kernel(Q, K, V, Wq, Wk, Wv, Wo):
    in_maps = make_in_maps(np.asarray(Q), np.asarray(K), np.asarray(V),
                           np.asarray(Wq), np.asarray(Wk), np.asarray(Wv),
                           np.asarray(Wo))
    res = run_on_hw(in_maps)
    out = np.zeros((N, DIM), dtype=np.float64)
    for r in res.results:
        den = r["den"].astype(np.float64).T.reshape(-1)  # [tile, row] -> query
        out += r["out"].astype(np.float64) / den[:, None]
    return out.astype(np.float32)


if __name__ == "__main__":
    rng = np.random.default_rng(0)
    inputs = {
        "Q": rng.standard_normal((N, DIM), dtype=np.float32),
        "K": rng.standard_normal((N, DIM), dtype=np.float32),
        "V": rng.standard_normal((N, DIM), dtype=np.float32),
        "Wq": rng.random((H, DIM, D), dtype=np.float32),
        "Wk": rng.random((H, DIM, D), dtype=np.float32),
        "Wv": rng.random((H, DIM, D), dtype=np.float32),
        "Wo": rng.random((DIM, DIM), dtype=np.float32),
    }
    out = kernel(**inputs)
    print(out.shape, out.dtype, np.abs(out).max())
